# revision 7
# baseline (speedup 1.0000x reference)
"""ContactMapLinear Trainium2 kernel.

res = tril((X @ P) @ (Q @ X^T), k=-1), X = features[0, 1:4097, :], 8-core SPMD.

Sharding: core c owns the interleaved seq rows {8r + c : r in [0, 512)} for the
row side (phases A, C) and the contiguous seq col block [512c, 512c+512) for
the col side (phase B). Row interleaving makes the strictly-lower-triangular
structure identical on every core, so the SPMD program statically skips the
upper-triangular column blocks:

  Phase A: AT_c = P^T @ Xrows_c^T    [1024, 512]   (A = X@P, stored transposed)
  Phase B: B_c  = Q   @ Xcols_c^T    [1024, 512]   (cols of B = Q @ X^T)
  AllGather B_c over 8 cores -> B    [1024, 4096]  (contiguous col blocks)
  Phase C: for row m-tile t (local rows 128t..128t+127, global rows
           1024t + 8r' + c), only col blocks j <= 2t+1 can be below the
           diagonal; block j == 2t, 2t+1 is masked with
           mask[r', q] = (q < 8r' + c), q = col offset within the 1024-block
           (independent of t). Unwritten output regions stay zero (outputs
           are pre-zeroed by the runtime).

All matmul inputs are bf16 (fp32 PSUM accumulation); output is fp32.
Host reassembles rows: out_full[c::8] = out_c.
"""

import sys

import ml_dtypes
import numpy as np

_TRN_REPO = "/opt/trn_rl_repo"
if _TRN_REPO not in sys.path:
    sys.path.insert(0, _TRN_REPO)

D = 4096          # seq length / feature dim
I = 1024          # inner dim
N_CORES = 8
R = D // N_CORES  # 512 seq rows per core
P = 128           # partitions
KT = D // P       # 32 feature k-tiles
IT = I // P       # 8 inner tiles
MT = R // P       # 4 row m-tiles per core
BF16 = ml_dtypes.bfloat16

_CACHE = {}


def _build(repeat: int = 1):
    import concourse.mybir as mybir
    import concourse.tile as tile
    from concourse import bacc

    dt = mybir.dt
    nc = bacc.Bacc("TRN2", target_bir_lowering=False, debug=False,
                   num_devices=N_CORES)

    xtr_in = nc.declare_dram_parameter("xtr", [D, R], dt.bfloat16, isOutput=False)
    xtc_in = nc.declare_dram_parameter("xtc", [D, R], dt.bfloat16, isOutput=False)
    p_in = nc.declare_dram_parameter("p", [D, I], dt.bfloat16, isOutput=False)
    qt_in = nc.declare_dram_parameter("qt", [D, I], dt.bfloat16, isOutput=False)
    mask_in = nc.declare_dram_parameter("mask", [P, I], dt.float32, isOutput=False)
    out = nc.declare_dram_parameter("out", [R, D], dt.float32, isOutput=True)

    xtr_ap = xtr_in.ap().rearrange("(ko ki) n -> ki ko n", ki=P)  # [128, 32, 512]
    xtc_ap = xtc_in.ap().rearrange("(ko ki) n -> ki ko n", ki=P)
    p_ap = p_in.ap().rearrange("(ko ki) m -> ki ko m", ki=P)      # [128, 32, 1024]
    qt_ap = qt_in.ap().rearrange("(ko ki) m -> ki ko m", ki=P)
    out_ap = out.ap().rearrange("(mo mi) n -> mi mo n", mi=P)     # [128, 4, 4096]

    with tile.TileContext(nc) as tc:
        with (
            tc.tile_pool(name="xt", bufs=1) as xt_pool,
            tc.tile_pool(name="w", bufs=3) as w_pool,
            tc.tile_pool(name="ab", bufs=1) as ab_pool,
            tc.tile_pool(name="bj", bufs=2) as bj_pool,
            tc.tile_pool(name="oc", bufs=4) as oc_pool,
            tc.tile_pool(name="msk", bufs=1) as msk_pool,
            tc.tile_pool(name="ps", bufs=1, space="PSUM") as ps_pool,
            tc.tile_pool(name="dram", bufs=1, space="DRAM") as dram_pool,
        ):
            for _rep in range(repeat):
                mask_sb = msk_pool.tile([P, I], dt.float32, name="mask",
                                        tag="mask")
                nc.sync.dma_start(out=mask_sb[:], in_=mask_in.ap())

                xtr_sb, xtc_sb = [], []
                for k in range(KT):
                    tr = xt_pool.tile([P, R], dt.bfloat16, name=f"xtr{k}",
                                      tag=f"xtr{k}")
                    nc.sync.dma_start(out=tr[:], in_=xtr_ap[:, k, :])
                    xtr_sb.append(tr)
                    tcol = xt_pool.tile([P, R], dt.bfloat16, name=f"xtc{k}",
                                        tag=f"xtc{k}")
                    nc.sync.dma_start(out=tcol[:], in_=xtc_ap[:, k, :])
                    xtc_sb.append(tcol)

                at_sb = ab_pool.tile([P, IT, R], dt.bfloat16, name="at", tag="at")
                b_sb = ab_pool.tile([P, IT, R], dt.bfloat16, name="b", tag="b")

                # Phases A and B: k-outer streaming of P / Q^T, 8 PSUM banks
                # (one per inner m-tile) accumulate across all 32 k-tiles.
                for w_ap, xs, dst in ((p_ap, xtr_sb, at_sb),
                                      (qt_ap, xtc_sb, b_sb)):
                    psums = [
                        ps_pool.tile([P, R], dt.float32, name=f"ps{m}",
                                     tag=f"ps{m}")
                        for m in range(IT)
                    ]
                    for k in range(KT):
                        w_sb = w_pool.tile([P, I], dt.bfloat16, name="w", tag="w")
                        nc.sync.dma_start(out=w_sb[:], in_=w_ap[:, k, :])
                        for m in range(IT):
                            nc.tensor.matmul(
                                psums[m][:],
                                lhsT=w_sb[:, m * P:(m + 1) * P],
                                rhs=xs[k][:],
                                start=(k == 0),
                                stop=(k == KT - 1),
                            )
                    for m in range(IT):
                        nc.any.tensor_copy(out=dst[:, m, :], in_=psums[m][:])

                # AllGather B across cores -> contiguous col blocks
                bloc = dram_pool.tile([P, IT, R], dt.bfloat16, name="bloc",
                                      tag="bloc")
                ball = dram_pool.tile([N_CORES, P, IT, R], dt.bfloat16,
                                      name="ball", tag="ball",
                                      addr_space="Shared")
                nc.sync.dma_start(out=bloc[:], in_=b_sb[:])
                nc.gpsimd.collective_compute(
                    "AllGather",
                    mybir.AluOpType.bypass,
                    replica_groups=[list(range(N_CORES))],
                    ins=[bloc.opt()],
                    outs=[ball.opt()],
                )

                # Phase C: S rows = AT^T @ B over the lower-triangular
                # staircase. Row m-tile t needs col blocks j <= 2t+1.
                for j in range(N_CORES):
                    bj = bj_pool.tile([P, IT, R], dt.bfloat16, name="bj",
                                      tag="bj")
                    nc.sync.dma_start(out=bj[:], in_=ball[j])
                    for t in range(j // 2, MT):
                        ps = ps_pool.tile([P, R], dt.float32, name=f"psc{t}",
                                          tag=f"ps{t}")
                        for k in range(IT):
                            nc.tensor.matmul(
                                ps[:],
                                lhsT=at_sb[:, k, t * P:(t + 1) * P],
                                rhs=bj[:, k, :],
                                start=(k == 0),
                                stop=(k == IT - 1),
                            )
                        ot = oc_pool.tile([P, R], dt.float32, name="oc",
                                          tag="oc")
                        if t == j // 2:  # diagonal block: strict-lower mask
                            half = (j % 2) * R
                            nc.vector.tensor_tensor(
                                ot[:], ps[:], mask_sb[:, half:half + R],
                                mybir.AluOpType.mult,
                            )
                        else:
                            nc.any.tensor_copy(out=ot[:], in_=ps[:])
                        nc.sync.dma_start(out=out_ap[:, t, j * R:(j + 1) * R],
                                          in_=ot[:])

    nc.compile()
    return nc


def _make_in_maps(features: np.ndarray, Pm: np.ndarray, Qm: np.ndarray):
    X = np.ascontiguousarray(features[0, 1:1 + D, :], dtype=np.float32)
    p_bf = np.ascontiguousarray(Pm, dtype=np.float32).astype(BF16)
    qt_bf = np.ascontiguousarray(Qm.astype(np.float32).T).astype(BF16)
    r_idx = np.arange(P)
    q_idx = np.arange(I)
    in_maps = []
    for c in range(N_CORES):
        xtr_c = np.ascontiguousarray(X[c::8, :].T).astype(BF16)
        xtc_c = np.ascontiguousarray(X[c * R:(c + 1) * R, :].T).astype(BF16)
        mask_c = (q_idx[None, :] < (8 * r_idx[:, None] + c)).astype(np.float32)
        in_maps.append({"xtr": xtr_c, "xtc": xtc_c, "p": p_bf, "qt": qt_bf,
                        "mask": mask_c})
    return in_maps


def kernel(features: np.ndarray, P: np.ndarray, Q: np.ndarray) -> np.ndarray:
    from concourse.bass_utils import run_bass_kernel_spmd

    if "nc" not in _CACHE:
        _CACHE["nc"] = _build()
    nc = _CACHE["nc"]

    in_maps = _make_in_maps(features, P, Q)
    res = run_bass_kernel_spmd(nc, in_maps, list(range(N_CORES)))
    out_full = np.empty((D, D), dtype=np.float32)
    for c in range(N_CORES):
        out_full[c::8] = res.results[c]["out"]
    return out_full
